# revision 15
# baseline (speedup 1.0000x reference)
"""Distributed Trainium2 kernel for nn_Attention (RMSNorm + QKV + RoPE +
causal SDPA + out-proj) over 8 NeuronCores.

v8 strategy (head-sharded QKV, deferred normalization): every core
receives the FULL x (transposed, bf16) and projects q/k/v for only ITS
two heads over all 4096 tokens -- identical FLOPs to a local-chunk/
all-heads split, but no projection AllToAll.

  phase 0: RMSNorm stats for the core's own 512-token chunk (from a
           small dedicated copy of its x columns, so the chain doesn't
           sit behind the 8MB x stream); 1/rms AllGather'ed (2KB).
           The gather posts at ~9us and completes -- including CC
           channel setup and launch skew -- entirely under phase 1.
  phase 1: q/k projections for the 2 heads over all tokens into one
           2-bank psum pair (single evict copy + swap set), RoPE at
           evict with RAW cos/sin tables; v projected directly
           TRANSPOSED (tokens on partitions).  Nothing here depends on
           the gather: 1/rms is applied afterwards in a short bulk
           pass (qT/kT scaled in place via a ones-matmul broadcast of
           the gathered 1/rms; vv scaled in place per key-block with
           per-partition scalars).  RoPE and per-token scaling
           commute, so this is exact.  norm_w is folded into the
           weights on the host; weights arrive pre-cast to bf16.
  phase 2: causal SDPA in S^T layout, batches interleaved; both heads'
           scores in one 2-bank psum pair, single 1024-wide exp for
           full blocks (ScalarE, no max subtraction); score matmuls
           emitted one key-block ahead of AV so PE never waits on exp;
           ones column in V rides the softmax denominator in the same
           psum tile; deferred division via partition_broadcast on
           GpSimd + one DVE multiply.
  A2A:     context head-sharded -> token-sharded (1MB/rank).
  phase 3: out-projection for the core's own 512-token chunk.
Host does layout-only prep (transpose+bf16 cast, head-column
permutation, norm_w fold, constant RoPE/mask tables) and final concat.
"""
import sys

sys.path.insert(0, "/opt/trn_rl_repo")

import numpy as np
import ml_dtypes
from contextlib import ExitStack

import concourse.bass as bass
import concourse.mybir as mybir
import concourse.tile as tile
from concourse import bacc
from concourse.bass_utils import run_bass_kernel_spmd

F32 = mybir.dt.float32
BF16 = mybir.dt.bfloat16

B, S, D, H, DH = 2, 2048, 1024, 16, 64
NC = 8
TOK = B * S            # 4096
CHUNK = TOK // NC      # 512
EPS = 1.1920929e-07
THETA = 10000.0
NKB = S // 128         # key blocks per batch: 16
QT = S // 512          # q tiles per batch: 4

_CACHE = {}
DEBUG = False


def _build():
    nc = bacc.Bacc("TRN2", target_bir_lowering=False, debug=False, num_devices=NC)

    xc_d = nc.dram_tensor("xc", [D, CHUNK], BF16, kind="ExternalInput")
    xtb_d = nc.dram_tensor("xtb", [D, TOK], BF16, kind="ExternalInput")
    wq_d = nc.dram_tensor("wqc", [D, 128], BF16, kind="ExternalInput")
    wk_d = nc.dram_tensor("wkc", [D, 128], BF16, kind="ExternalInput")
    wv_d = nc.dram_tensor("wvc", [D, 128], BF16, kind="ExternalInput")
    wo_d = nc.dram_tensor("wo", [D, D], BF16, kind="ExternalInput")
    cos_d = nc.dram_tensor("cosb", [128, TOK], BF16, kind="ExternalInput")
    sin_d = nc.dram_tensor("sinb", [128, TOK], BF16, kind="ExternalInput")
    msk_d = nc.dram_tensor("dmask", [128, 128], BF16, kind="ExternalInput")
    out_d = nc.dram_tensor("out", [CHUNK, D], F32, kind="ExternalOutput")

    if DEBUG:
        qTd = nc.dram_tensor("qTd", [128, TOK], BF16, kind="ExternalOutput")
        kTd = nc.dram_tensor("kTd", [128, TOK], BF16, kind="ExternalOutput")
        vvd = nc.dram_tensor("vvd", [128, 32 * 130], BF16, kind="ExternalOutput")
        ctxd = nc.dram_tensor("ctxd", [128, TOK], BF16, kind="ExternalOutput")
    ag_in = nc.dram_tensor("ag_in", [1, CHUNK], F32)
    ag_out = nc.dram_tensor("ag_out", [1, TOK], F32)
    a2a_in = nc.dram_tensor("a2a_in", [NC * 128, CHUNK], BF16)
    a2a_out = nc.dram_tensor("a2a_out", [NC * 128, CHUNK], BF16)

    with tile.TileContext(nc) as tc, ExitStack() as ctx:
        pp = ctx.enter_context(tc.tile_pool(name="persist", bufs=1))

        # ---- persistent tiles ----
        qT = pp.tile([128, TOK], BF16, tag="qT")
        kT = pp.tile([128, TOK], BF16, tag="kT")
        # vv[:, blk, :] = [h0 dims 0:64 | ones | h1 dims 65:129 | ones]
        vv = pp.tile([128, B * NKB, 130], BF16, tag="vv")
        cosS = pp.tile([128, TOK], BF16, tag="cosS")
        sinS = pp.tile([128, TOK], BF16, tag="sinS")
        ctx_sb = pp.tile([128, TOK], BF16, tag="ctx_sb")
        wq_sb = pp.tile([128, 8, 128], BF16, tag="wq_sb")
        wk_sb = pp.tile([128, 8, 128], BF16, tag="wk_sb")
        wv_sb = pp.tile([128, 8, 128], BF16, tag="wv_sb")
        wo_sb = pp.tile([128, 8, 1024], BF16, tag="wo_sb")
        inv_all = pp.tile([1, TOK], F32, tag="inv_all")
        invT = pp.tile([128, B * NKB], F32, tag="invT")
        dmaskT = pp.tile([128, 128], BF16, tag="dmaskT")
        ones128 = pp.tile([128, 1], BF16, tag="ones128")
        ones1 = pp.tile([1, 128], BF16, tag="ones1")

        nc.vector.memset(ones128, 1.0)
        nc.vector.memset(ones1, 1.0)
        nc.vector.memset(vv[:, :, 64:65], 1.0)
        nc.vector.memset(vv[:, :, 129:130], 1.0)

        xs_cm = tc.tile_pool(name="xspool", bufs=1)
        xs_pool = xs_cm.__enter__()
        xs = xs_pool.tile([128, 8, TOK], BF16, tag="xs")
        xcs = xs_pool.tile([128, 8, CHUNK], BF16, tag="xcs")

        # ---- input DMAs (big 3D-AP transfers; issue in consumption order)
        nc.sync.dma_start(
            out=xcs, in_=xc_d.ap().rearrange("(k p) t -> p k t", p=128))
        nc.scalar.dma_start(
            out=wq_sb, in_=wq_d.ap().rearrange("(k p) d -> p k d", p=128))
        nc.scalar.dma_start(
            out=wk_sb, in_=wk_d.ap().rearrange("(k p) d -> p k d", p=128))
        nc.scalar.dma_start(
            out=wv_sb, in_=wv_d.ap().rearrange("(k p) d -> p k d", p=128))
        for tt in range(NC):
            sl = slice(tt * CHUNK, (tt + 1) * CHUNK)
            nc.sync.dma_start(
                out=xs[:, :, sl],
                in_=xtb_d.ap()[:, sl].rearrange("(k p) t -> p k t", p=128))
        nc.scalar.dma_start(out=cosS, in_=cos_d[:, :])
        nc.scalar.dma_start(out=sinS, in_=sin_d[:, :])
        nc.scalar.dma_start(out=dmaskT, in_=msk_d[:, :])
        nc.scalar.dma_start(
            out=wo_sb, in_=wo_d.ap().rearrange("(k p) d -> p k d", p=128))

        # ---- phase 0: RMSNorm stats of own chunk + AllGather of 1/rms ----
        with tc.tile_pool(name="rms", bufs=2) as rms_pool, \
             tc.tile_pool(name="psrms", bufs=1, space="PSUM") as psrms:
            ssq = psrms.tile([1, CHUNK], F32, tag="ssq")
            for kt in range(8):
                xsq = rms_pool.tile([128, CHUNK], BF16, tag="xsq")
                nc.vector.tensor_mul(xsq, xcs[:, kt, :], xcs[:, kt, :])
                nc.tensor.matmul(ssq, ones128, xsq, start=(kt == 0), stop=(kt == 7))
            eps_t = rms_pool.tile([1, 1], F32, tag="eps_t")
            nc.vector.memset(eps_t, float(EPS))
            rstd = rms_pool.tile([1, CHUNK], F32, tag="rstd")
            nc.scalar.activation(rstd, ssq, mybir.ActivationFunctionType.Sqrt,
                                 bias=eps_t[0:1, 0:1], scale=1.0 / D)
            inv = rms_pool.tile([1, CHUNK], F32, tag="inv")
            nc.vector.reciprocal_approx_fast(out=inv, in_=rstd)
            nc.scalar.dma_start(out=ag_in[:, :], in_=inv)

        nc.gpsimd.collective_compute(
            "AllGather", mybir.AluOpType.bypass,
            replica_groups=[list(range(NC))],
            ins=[ag_in.ap().opt()], outs=[ag_out.ap().opt()])
        nc.scalar.dma_start(out=inv_all, in_=ag_out[:, :])
        nc.scalar.dma_start(
            out=invT, in_=ag_out.ap().rearrange("a (k p) -> p (a k)", p=128))

        # ---- phase 1: QKV + RoPE (raw tables; scaling deferred) ----
        with tc.tile_pool(name="pstage", bufs=4) as pstage, \
             tc.tile_pool(name="psqk", bufs=3, space="PSUM") as psqk, \
             tc.tile_pool(name="psv", bufs=1, space="PSUM") as psv, \
             tc.tile_pool(name="psrb", bufs=1, space="PSUM") as psrb, \
             tc.tile_pool(name="rbst", bufs=2) as rbst:
            for tt in range(NC):
                sl = slice(tt * CHUNK, (tt + 1) * CHUNK)
                # q/k projections into one 2-bank psum pair
                acc = psqk.tile([128, 1024], F32, tag="acc")
                for wsb, half in ((wq_sb, 0), (wk_sb, 1)):
                    for kt in range(8):
                        nc.tensor.matmul(
                            acc[:, half * 512 : half * 512 + 512],
                            wsb[:, kt, :], xs[:, kt, sl],
                            start=(kt == 0), stop=(kt == 7))
                t = pstage.tile([128, 1024], BF16, tag="t")
                nc.scalar.copy(t, acc)
                sw = pstage.tile([128, 1024], BF16, tag="sw")
                for a, b2 in ((0, 32), (64, 96)):
                    nc.sync.dma_start(out=sw[a : a + 32, :], in_=t[b2 : b2 + 32, :])
                    nc.sync.dma_start(out=sw[b2 : b2 + 32, :], in_=t[a : a + 32, :])
                t1 = pstage.tile([128, 1024], BF16, tag="t1")
                for half, dst in ((0, qT), (1, kT)):
                    hs = slice(half * 512, half * 512 + 512)
                    nc.vector.tensor_mul(t1[:, hs], t[:, hs], cosS[:, sl])
                    nc.vector.tensor_mul(sw[:, hs], sw[:, hs], sinS[:, sl])
                    nc.vector.tensor_add(dst[:, sl], t1[:, hs], sw[:, hs])
                # v transposed directly: lhsT = x token-block (stationary)
                for vb in range(4):
                    blk = tt * 4 + vb
                    c0 = tt * CHUNK + vb * 128
                    accv = psv.tile([128, 128], F32, tag="accv")
                    for kt in range(8):
                        nc.tensor.matmul(accv, xs[:, kt, c0 : c0 + 128],
                                         wv_sb[:, kt, :],
                                         start=(kt == 0), stop=(kt == 7))
                    nc.vector.tensor_copy(vv[:, blk, 0:64], accv[:, 0:64])
                    nc.vector.tensor_copy(vv[:, blk, 65:129], accv[:, 64:128])

            # ---- deferred 1/rms application (gated on the AllGather, which
            # completed long ago under phase 1) ----
            for tt in range(NC):
                sl = slice(tt * CHUNK, (tt + 1) * CHUNK)
                invb = rbst.tile([1, CHUNK], BF16, tag="invb")
                nc.vector.tensor_copy(invb, inv_all[0:1, sl])
                rb = psrb.tile([128, CHUNK], F32, tag="rb")
                nc.tensor.matmul(rb, ones1, invb, start=True, stop=True)
                nc.vector.tensor_mul(qT[:, sl], qT[:, sl], rb)
                nc.vector.tensor_mul(kT[:, sl], kT[:, sl], rb)
                for vb in range(4):
                    blk = tt * 4 + vb
                    nc.gpsimd.tensor_scalar_mul(
                        vv[:, blk, 0:64], vv[:, blk, 0:64], invT[:, blk : blk + 1])
                    nc.gpsimd.tensor_scalar_mul(
                        vv[:, blk, 65:129], vv[:, blk, 65:129], invT[:, blk : blk + 1])

        xs_cm.__exit__(None, None, None)
        vvf = vv.rearrange("p blk c -> p (blk c)")

        # ---- phase 2: SDPA (batches interleaved; scores one block ahead) ----
        with tc.tile_pool(name="pexp", bufs=6) as pexp, \
             tc.tile_pool(name="cnorm", bufs=2) as cnorm, \
             tc.tile_pool(name="ps4", bufs=2, space="PSUM") as ps4, \
             tc.tile_pool(name="ps4c", bufs=1, space="PSUM") as ps4c:
            for step in range(B * QT):
                b, j = step % B, step // B
                base = b * S
                ctxp = {0: ps4c.tile([65, 512], F32, name=f"ctxA{b}", tag=f"ctxA{b}"),
                        1: ps4c.tile([65, 512], F32, name=f"ctxB{b}", tag=f"ctxB{b}")}
                nkb = 4 * (j + 1)

                def params(kb):
                    m = kb - 4 * j
                    c0 = 128 * m if m >= 0 else 0
                    return m, c0, 512 - c0

                def emit_sc(kb):
                    m, c0, w = params(kb)
                    qcol0 = base + 512 * j + c0
                    koff = base + kb * 128
                    sc = ps4.tile([128, 1024], F32, name="sc", tag="sc")
                    for hi, r0 in ((0, 0), (1, 64)):
                        nc.tensor.matmul(
                            sc[:, hi * 512 + c0 : hi * 512 + 512],
                            kT[r0 : r0 + 64, koff : koff + 128],
                            qT[r0 : r0 + 64, qcol0 : qcol0 + w],
                            start=True, stop=True)
                    return sc

                sc_cur = emit_sc(0)
                for kb in range(nkb):
                    m, c0, w = params(kb)
                    sc = sc_cur
                    p = pexp.tile([128, 1024], BF16, name="p", tag="p")
                    if m >= 0:
                        for hi in (0, 1):
                            nc.scalar.activation(
                                p[:, hi * 512 + c0 : hi * 512 + 512],
                                sc[:, hi * 512 + c0 : hi * 512 + 512],
                                mybir.ActivationFunctionType.Exp, scale=0.125)
                    else:
                        nc.scalar.activation(
                            p, sc, mybir.ActivationFunctionType.Exp, scale=0.125)
                    if kb + 1 < nkb:
                        sc_cur = emit_sc(kb + 1)
                    if m >= 0:
                        for hi in (0, 1):
                            nc.vector.tensor_mul(
                                p[:, hi * 512 + c0 : hi * 512 + c0 + 128],
                                p[:, hi * 512 + c0 : hi * 512 + c0 + 128], dmaskT)
                    for hi in (0, 1):
                        vcol = (b * NKB + kb) * 130 + hi * 65
                        nc.tensor.matmul(
                            ctxp[hi][:, c0:512],
                            vvf[:, vcol : vcol + 65],
                            p[:, hi * 512 + c0 : hi * 512 + 512],
                            start=(kb == 0), stop=(kb == nkb - 1),
                            skip_group_check=True)
                # normalize: ctx / denom (denom = row 64 of ctx psum)
                cch = b * QT + j
                csl = slice(cch * CHUNK, (cch + 1) * CHUNK)
                for hi, r0 in ((0, 0), (1, 64)):
                    den_s = cnorm.tile([1, 512], F32, tag="den_s")
                    nc.vector.tensor_copy(den_s, ctxp[hi][64:65, :])
                    rec = cnorm.tile([1, 512], F32, tag="rec")
                    nc.vector.reciprocal_approx_fast(out=rec, in_=den_s)
                    recb = cnorm.tile([1, 512], BF16, tag="recb")
                    nc.vector.tensor_copy(recb, rec)
                    bcs = cnorm.tile([64, 512], BF16, tag="bcs")
                    nc.gpsimd.partition_broadcast(bcs[:, :], recb[0:1, :])
                    nc.vector.tensor_mul(
                        ctx_sb[r0 : r0 + 64, csl], ctxp[hi][0:64, :], bcs)
                nc.sync.dma_start(
                    out=a2a_in[cch * 128 : (cch + 1) * 128, :], in_=ctx_sb[:, csl])

        if DEBUG:
            nc.sync.dma_start(out=qTd[:, :], in_=qT)
            nc.sync.dma_start(out=kTd[:, :], in_=kT)
            nc.sync.dma_start(out=vvd[:, :], in_=vv.rearrange("p b c -> p (b c)"))
            nc.sync.dma_start(out=ctxd[:, :], in_=ctx_sb)

        # ---- A2A: head-sharded ctx -> token-sharded ctx ----
        nc.gpsimd.collective_compute(
            "AllToAll", mybir.AluOpType.bypass,
            replica_groups=[list(range(NC))],
            ins=[a2a_in.ap().opt()], outs=[a2a_out.ap().opt()])

        # ---- phase 3: out-projection on own token chunk ----
        with tc.tile_pool(name="ctxgp", bufs=1) as ctxgp, \
             tc.tile_pool(name="outp", bufs=3) as outp, \
             tc.tile_pool(name="ps6", bufs=2, space="PSUM") as ps6:
            ctxg = ctxgp.tile([128, 8, CHUNK], BF16, tag="ctxg")
            for tl in range(4):
                nc.scalar.dma_start(
                    out=ctxg[:, :, tl * 128 : (tl + 1) * 128],
                    in_=a2a_out.ap()[:, tl * 128 : (tl + 1) * 128]
                        .rearrange("(cb p) t -> p cb t", p=128))
            for tl in range(4):
                pso = {nh: ps6.tile([128, 512], F32, name=f"op{nh}", tag=f"op{nh}")
                       for nh in range(2)}
                for nh in range(2):
                    for cb in range(8):
                        nc.tensor.matmul(
                            pso[nh],
                            ctxg[:, cb, tl * 128 : (tl + 1) * 128],
                            wo_sb[:, cb, nh * 512 : (nh + 1) * 512],
                            start=(cb == 0), stop=(cb == 7))
                ost = outp.tile([128, 1024], F32, tag="ost")
                nc.scalar.copy(ost[:, 0:512], pso[0])
                nc.scalar.copy(ost[:, 512:1024], pso[1])
                nc.sync.dma_start(out=out_d[tl * 128 : (tl + 1) * 128, :], in_=ost)

    nc.compile()
    return nc


def _head_cols(h, deinterleave):
    base = h * DH
    if deinterleave:
        return np.concatenate([base + np.arange(0, DH, 2), base + np.arange(1, DH, 2)])
    return base + np.arange(DH)


def _make_tables():
    inv_freq = 1.0 / (THETA ** (np.arange(0, DH, 2) / DH))   # [32]
    ang = np.arange(S)[:, None] * inv_freq[None, :]          # [2048, 32]
    ch = np.cos(ang).T.astype(np.float32)                    # [32, 2048]
    sh = np.sin(ang).T.astype(np.float32)
    cosb = np.tile(np.concatenate([ch, ch, ch, ch], axis=0), (1, B))
    sinb = np.tile(np.concatenate([-sh, sh, -sh, sh], axis=0), (1, B))
    kk, qq = np.meshgrid(np.arange(128), np.arange(128), indexing="ij")
    dmask = (kk <= qq).astype(np.float32)
    bf = ml_dtypes.bfloat16
    return cosb.astype(bf), sinb.astype(bf), dmask.astype(bf)


def _in_maps(inputs):
    bf = ml_dtypes.bfloat16
    x = np.ascontiguousarray(inputs["x"], dtype=np.float32)
    norm_w = np.asarray(inputs["norm_w"], dtype=np.float32)
    wq = np.asarray(inputs["wq"], dtype=np.float32) * norm_w[:, None]
    wk = np.asarray(inputs["wk"], dtype=np.float32) * norm_w[:, None]
    wv = np.asarray(inputs["wv"], dtype=np.float32) * norm_w[:, None]
    wo = np.ascontiguousarray(inputs["wo"], dtype=np.float32).astype(bf)

    xT = np.ascontiguousarray(x.reshape(TOK, D).T.astype(bf))  # [1024, 4096]
    cosb, sinb, dmask = _make_tables()

    maps = []
    for c in range(NC):
        qcols = np.concatenate([_head_cols(2 * c, True), _head_cols(2 * c + 1, True)])
        vcols = np.concatenate([_head_cols(2 * c, False), _head_cols(2 * c + 1, False)])
        maps.append({
            "xc": np.ascontiguousarray(xT[:, c * CHUNK : (c + 1) * CHUNK]),
            "xtb": xT,
            "wqc": np.ascontiguousarray(wq[:, qcols].astype(bf)),
            "wkc": np.ascontiguousarray(wk[:, qcols].astype(bf)),
            "wvc": np.ascontiguousarray(wv[:, vcols].astype(bf)),
            "wo": wo,
            "cosb": cosb,
            "sinb": sinb,
            "dmask": dmask,
        })
    return maps


def _run(inputs, trace=False):
    if "nc" not in _CACHE:
        _CACHE["nc"] = _build()
    nc = _CACHE["nc"]
    res = run_bass_kernel_spmd(nc, _in_maps(inputs), core_ids=list(range(NC)),
                               trace=trace)
    chunks = [res.results[c]["out"] for c in range(NC)]
    out = np.concatenate(chunks, axis=0).reshape(B, S, D).astype(np.float32)
    return out, res


def kernel(**inputs) -> np.ndarray:
    out, _ = _run(inputs, trace=False)
    return out


# revision 16
# speedup vs baseline: 1.1100x; 1.1100x over previous
"""Distributed Trainium2 kernel for nn_Attention (RMSNorm + QKV + RoPE +
causal SDPA + out-proj) over 8 NeuronCores.

v8 strategy (head-sharded QKV, deferred normalization): every core
receives the FULL x (transposed, bf16) and projects q/k/v for only ITS
two heads over all 4096 tokens -- identical FLOPs to a local-chunk/
all-heads split, but no projection AllToAll.

  phase 0: RMSNorm stats for the core's own 512-token chunk (from a
           small dedicated copy of its x columns, so the chain doesn't
           sit behind the 8MB x stream); 1/rms AllGather'ed (2KB).
           The gather posts at ~9us and completes -- including CC
           channel setup and launch skew -- entirely under phase 1.
  phase 1: q/k projections for the 2 heads over all tokens into one
           2-bank psum pair (single evict copy + swap set), RoPE at
           evict with RAW cos/sin tables; v projected directly
           TRANSPOSED (tokens on partitions).  Nothing here depends on
           the gather: 1/rms is applied afterwards in a short bulk
           pass (qT/kT scaled in place via a ones-matmul broadcast of
           the gathered 1/rms; vv scaled in place per key-block with
           per-partition scalars).  RoPE and per-token scaling
           commute, so this is exact.  norm_w is folded into the
           weights on the host; weights arrive pre-cast to bf16.
  phase 2: causal SDPA in S^T layout, batches interleaved; both heads'
           scores in one 2-bank psum pair, single 1024-wide exp for
           full blocks (ScalarE, no max subtraction); score matmuls
           emitted one key-block ahead of AV so PE never waits on exp;
           ones column in V rides the softmax denominator in the same
           psum tile; deferred division via partition_broadcast on
           GpSimd + one DVE multiply.
  A2A:     context head-sharded -> token-sharded (1MB/rank).
  phase 3: out-projection for the core's own 512-token chunk.
Host does layout-only prep (transpose+bf16 cast, head-column
permutation, norm_w fold, constant RoPE/mask tables) and final concat.
"""
import sys

sys.path.insert(0, "/opt/trn_rl_repo")

import numpy as np
import ml_dtypes
from contextlib import ExitStack

import concourse.bass as bass
import concourse.mybir as mybir
import concourse.tile as tile
from concourse import bacc
from concourse.bass_utils import run_bass_kernel_spmd

F32 = mybir.dt.float32
BF16 = mybir.dt.bfloat16

B, S, D, H, DH = 2, 2048, 1024, 16, 64
NC = 8
TOK = B * S            # 4096
CHUNK = TOK // NC      # 512
EPS = 1.1920929e-07
THETA = 10000.0
NKB = S // 128         # key blocks per batch: 16
QT = S // 512          # q tiles per batch: 4

_CACHE = {}
DEBUG = False


def _build():
    nc = bacc.Bacc("TRN2", target_bir_lowering=False, debug=False, num_devices=NC)

    xc_d = nc.dram_tensor("xc", [D, CHUNK], BF16, kind="ExternalInput")
    xtb_d = nc.dram_tensor("xtb", [D, TOK], BF16, kind="ExternalInput")
    wq_d = nc.dram_tensor("wqc", [D, 128], BF16, kind="ExternalInput")
    wk_d = nc.dram_tensor("wkc", [D, 128], BF16, kind="ExternalInput")
    wv_d = nc.dram_tensor("wvc", [D, 128], BF16, kind="ExternalInput")
    wo_d = nc.dram_tensor("wo", [D, D], BF16, kind="ExternalInput")
    cos_d = nc.dram_tensor("cosb", [128, TOK], BF16, kind="ExternalInput")
    sin_d = nc.dram_tensor("sinb", [128, TOK], BF16, kind="ExternalInput")
    msk_d = nc.dram_tensor("dmask", [128, 128], BF16, kind="ExternalInput")
    out_d = nc.dram_tensor("out", [CHUNK, D], F32, kind="ExternalOutput")

    if DEBUG:
        qTd = nc.dram_tensor("qTd", [128, TOK], BF16, kind="ExternalOutput")
        kTd = nc.dram_tensor("kTd", [128, TOK], BF16, kind="ExternalOutput")
        vvd = nc.dram_tensor("vvd", [128, 32 * 130], BF16, kind="ExternalOutput")
        ctxd = nc.dram_tensor("ctxd", [128, TOK], BF16, kind="ExternalOutput")
    ag_in = nc.dram_tensor("ag_in", [1, CHUNK], F32)
    ag_out = nc.dram_tensor("ag_out", [1, TOK], F32)
    a2a_in = nc.dram_tensor("a2a_in", [NC * 128, CHUNK], BF16)
    a2a_out = nc.dram_tensor("a2a_out", [NC * 128, CHUNK], BF16)

    with tile.TileContext(nc) as tc, ExitStack() as ctx:
        pp = ctx.enter_context(tc.tile_pool(name="persist", bufs=1))

        # ---- persistent tiles ----
        qT = pp.tile([128, TOK], BF16, tag="qT")
        kT = pp.tile([128, TOK], BF16, tag="kT")
        # vv[:, blk, :] = [h0 dims 0:64 | ones | h1 dims 65:129 | ones]
        vv = pp.tile([128, B * NKB, 130], BF16, tag="vv")
        cosS = pp.tile([128, TOK], BF16, tag="cosS")
        sinS = pp.tile([128, TOK], BF16, tag="sinS")
        ctx_sb = pp.tile([128, TOK], BF16, tag="ctx_sb")
        wq_sb = pp.tile([128, 8, 128], BF16, tag="wq_sb")
        wk_sb = pp.tile([128, 8, 128], BF16, tag="wk_sb")
        wv_sb = pp.tile([128, 8, 128], BF16, tag="wv_sb")
        wo_sb = pp.tile([128, 8, 1024], BF16, tag="wo_sb")
        inv_all = pp.tile([1, TOK], F32, tag="inv_all")
        invT = pp.tile([128, B * NKB], F32, tag="invT")
        dmaskT = pp.tile([128, 128], BF16, tag="dmaskT")
        ones128 = pp.tile([128, 1], BF16, tag="ones128")
        ones1 = pp.tile([1, 128], BF16, tag="ones1")

        nc.vector.memset(ones128, 1.0)
        nc.vector.memset(ones1, 1.0)
        nc.vector.memset(vv[:, :, 64:65], 1.0)
        nc.vector.memset(vv[:, :, 129:130], 1.0)

        xs_cm = tc.tile_pool(name="xspool", bufs=1)
        xs_pool = xs_cm.__enter__()
        xs = xs_pool.tile([128, 8, TOK], BF16, tag="xs")
        xcs = xs_pool.tile([128, 8, CHUNK], BF16, tag="xcs")

        # ---- input DMAs (big 3D-AP transfers; issue in consumption order)
        nc.sync.dma_start(
            out=xcs, in_=xc_d.ap().rearrange("(k p) t -> p k t", p=128))
        nc.scalar.dma_start(
            out=wq_sb, in_=wq_d.ap().rearrange("(k p) d -> p k d", p=128))
        nc.scalar.dma_start(
            out=wk_sb, in_=wk_d.ap().rearrange("(k p) d -> p k d", p=128))
        nc.scalar.dma_start(
            out=wv_sb, in_=wv_d.ap().rearrange("(k p) d -> p k d", p=128))
        for tt in range(NC):
            sl = slice(tt * CHUNK, (tt + 1) * CHUNK)
            nc.sync.dma_start(
                out=xs[:, :, sl],
                in_=xtb_d.ap()[:, sl].rearrange("(k p) t -> p k t", p=128))
        nc.scalar.dma_start(out=cosS, in_=cos_d[:, :])
        nc.scalar.dma_start(out=sinS, in_=sin_d[:, :])
        nc.scalar.dma_start(out=dmaskT, in_=msk_d[:, :])
        nc.scalar.dma_start(
            out=wo_sb, in_=wo_d.ap().rearrange("(k p) d -> p k d", p=128))

        # ---- phase 0: RMSNorm stats of own chunk + AllGather of 1/rms ----
        with tc.tile_pool(name="rms", bufs=2) as rms_pool, \
             tc.tile_pool(name="psrms", bufs=1, space="PSUM") as psrms:
            ssq = psrms.tile([1, CHUNK], F32, tag="ssq")
            for kt in range(8):
                xsq = rms_pool.tile([128, CHUNK], BF16, tag="xsq")
                nc.vector.tensor_mul(xsq, xcs[:, kt, :], xcs[:, kt, :])
                nc.tensor.matmul(ssq, ones128, xsq, start=(kt == 0), stop=(kt == 7))
            eps_t = rms_pool.tile([1, 1], F32, tag="eps_t")
            nc.vector.memset(eps_t, float(EPS))
            rstd = rms_pool.tile([1, CHUNK], F32, tag="rstd")
            nc.scalar.activation(rstd, ssq, mybir.ActivationFunctionType.Sqrt,
                                 bias=eps_t[0:1, 0:1], scale=1.0 / D)
            inv = rms_pool.tile([1, CHUNK], F32, tag="inv")
            nc.vector.reciprocal_approx_fast(out=inv, in_=rstd)
            nc.scalar.dma_start(out=ag_in[:, :], in_=inv)

        nc.gpsimd.collective_compute(
            "AllGather", mybir.AluOpType.bypass,
            replica_groups=[list(range(NC))],
            ins=[ag_in.ap().opt()], outs=[ag_out.ap().opt()])
        nc.scalar.dma_start(out=inv_all, in_=ag_out[:, :])
        nc.scalar.dma_start(
            out=invT, in_=ag_out.ap().rearrange("a (k p) -> p (a k)", p=128))

        # ---- phase 1: QKV + RoPE (raw tables; scaling deferred) ----
        with tc.tile_pool(name="pstage", bufs=4) as pstage, \
             tc.tile_pool(name="psqk", bufs=3, space="PSUM") as psqk, \
             tc.tile_pool(name="psv", bufs=1, space="PSUM") as psv, \
             tc.tile_pool(name="psrb", bufs=1, space="PSUM") as psrb, \
             tc.tile_pool(name="rbst", bufs=2) as rbst:
            for tt in range(NC):
                sl = slice(tt * CHUNK, (tt + 1) * CHUNK)
                # q/k projections into one 2-bank psum pair
                acc = psqk.tile([128, 1024], F32, tag="acc")
                for wsb, half in ((wq_sb, 0), (wk_sb, 1)):
                    for kt in range(8):
                        nc.tensor.matmul(
                            acc[:, half * 512 : half * 512 + 512],
                            wsb[:, kt, :], xs[:, kt, sl],
                            start=(kt == 0), stop=(kt == 7))
                t = pstage.tile([128, 1024], BF16, tag="t")
                nc.scalar.copy(t, acc)
                sw = pstage.tile([128, 1024], BF16, tag="sw")
                for a, b2 in ((0, 32), (64, 96)):
                    nc.sync.dma_start(out=sw[a : a + 32, :], in_=t[b2 : b2 + 32, :])
                    nc.sync.dma_start(out=sw[b2 : b2 + 32, :], in_=t[a : a + 32, :])
                t1 = pstage.tile([128, 1024], BF16, tag="t1")
                for half, dst in ((0, qT), (1, kT)):
                    hs = slice(half * 512, half * 512 + 512)
                    nc.vector.tensor_mul(t1[:, hs], t[:, hs], cosS[:, sl])
                    nc.vector.tensor_mul(sw[:, hs], sw[:, hs], sinS[:, sl])
                    nc.vector.tensor_add(dst[:, sl], t1[:, hs], sw[:, hs])
                # v transposed directly: lhsT = x token-block (stationary)
                for vb in range(4):
                    blk = tt * 4 + vb
                    c0 = tt * CHUNK + vb * 128
                    accv = psv.tile([128, 128], F32, tag="accv")
                    for kt in range(8):
                        nc.tensor.matmul(accv, xs[:, kt, c0 : c0 + 128],
                                         wv_sb[:, kt, :],
                                         start=(kt == 0), stop=(kt == 7))
                    nc.vector.tensor_copy(vv[:, blk, 0:64], accv[:, 0:64])
                    nc.vector.tensor_copy(vv[:, blk, 65:129], accv[:, 64:128])

            # ---- deferred 1/rms application (gated on the AllGather, which
            # completed long ago under phase 1) ----
            for tt in range(NC):
                sl = slice(tt * CHUNK, (tt + 1) * CHUNK)
                invb = rbst.tile([1, CHUNK], BF16, tag="invb")
                nc.vector.tensor_copy(invb, inv_all[0:1, sl])
                rb = psrb.tile([128, CHUNK], F32, tag="rb")
                nc.tensor.matmul(rb, ones1, invb, start=True, stop=True)
                nc.vector.tensor_mul(qT[:, sl], qT[:, sl], rb)
                nc.vector.tensor_mul(kT[:, sl], kT[:, sl], rb)
                for vb in range(4):
                    blk = tt * 4 + vb
                    nc.vector.tensor_scalar_mul(
                        vv[:, blk, 0:64], vv[:, blk, 0:64], invT[:, blk : blk + 1])
                    nc.vector.tensor_scalar_mul(
                        vv[:, blk, 65:129], vv[:, blk, 65:129], invT[:, blk : blk + 1])

        xs_cm.__exit__(None, None, None)
        vvf = vv.rearrange("p blk c -> p (blk c)")

        # ---- phase 2: SDPA (batches interleaved; scores one block ahead) ----
        with tc.tile_pool(name="pexp", bufs=6) as pexp, \
             tc.tile_pool(name="cnorm", bufs=2) as cnorm, \
             tc.tile_pool(name="ps4", bufs=2, space="PSUM") as ps4, \
             tc.tile_pool(name="ps4c", bufs=1, space="PSUM") as ps4c:
            for step in range(B * QT):
                b, j = step % B, step // B
                base = b * S
                ctxp = {0: ps4c.tile([65, 512], F32, name=f"ctxA{b}", tag=f"ctxA{b}"),
                        1: ps4c.tile([65, 512], F32, name=f"ctxB{b}", tag=f"ctxB{b}")}
                nkb = 4 * (j + 1)

                def params(kb):
                    m = kb - 4 * j
                    c0 = 128 * m if m >= 0 else 0
                    return m, c0, 512 - c0

                def emit_sc(kb):
                    m, c0, w = params(kb)
                    qcol0 = base + 512 * j + c0
                    koff = base + kb * 128
                    sc = ps4.tile([128, 1024], F32, name="sc", tag="sc")
                    for hi, r0 in ((0, 0), (1, 64)):
                        nc.tensor.matmul(
                            sc[:, hi * 512 + c0 : hi * 512 + 512],
                            kT[r0 : r0 + 64, koff : koff + 128],
                            qT[r0 : r0 + 64, qcol0 : qcol0 + w],
                            start=True, stop=True)
                    return sc

                sc_cur = emit_sc(0)
                for kb in range(nkb):
                    m, c0, w = params(kb)
                    sc = sc_cur
                    p = pexp.tile([128, 1024], BF16, name="p", tag="p")
                    if m >= 0:
                        for hi in (0, 1):
                            nc.scalar.activation(
                                p[:, hi * 512 + c0 : hi * 512 + 512],
                                sc[:, hi * 512 + c0 : hi * 512 + 512],
                                mybir.ActivationFunctionType.Exp, scale=0.125)
                    else:
                        nc.scalar.activation(
                            p, sc, mybir.ActivationFunctionType.Exp, scale=0.125)
                    if kb + 1 < nkb:
                        sc_cur = emit_sc(kb + 1)
                    if m >= 0:
                        for hi in (0, 1):
                            nc.vector.tensor_mul(
                                p[:, hi * 512 + c0 : hi * 512 + c0 + 128],
                                p[:, hi * 512 + c0 : hi * 512 + c0 + 128], dmaskT)
                    for hi in (0, 1):
                        vcol = (b * NKB + kb) * 130 + hi * 65
                        nc.tensor.matmul(
                            ctxp[hi][:, c0:512],
                            vvf[:, vcol : vcol + 65],
                            p[:, hi * 512 + c0 : hi * 512 + 512],
                            start=(kb == 0), stop=(kb == nkb - 1),
                            skip_group_check=True)
                # normalize: ctx / denom (denom = row 64 of ctx psum)
                cch = b * QT + j
                csl = slice(cch * CHUNK, (cch + 1) * CHUNK)
                for hi, r0 in ((0, 0), (1, 64)):
                    den_s = cnorm.tile([1, 512], F32, tag="den_s")
                    nc.vector.tensor_copy(den_s, ctxp[hi][64:65, :])
                    rec = cnorm.tile([1, 512], F32, tag="rec")
                    nc.vector.reciprocal_approx_fast(out=rec, in_=den_s)
                    recb = cnorm.tile([1, 512], BF16, tag="recb")
                    nc.vector.tensor_copy(recb, rec)
                    bcs = cnorm.tile([64, 512], BF16, tag="bcs")
                    nc.gpsimd.partition_broadcast(bcs[:, :], recb[0:1, :])
                    nc.vector.tensor_mul(
                        ctx_sb[r0 : r0 + 64, csl], ctxp[hi][0:64, :], bcs)
                nc.sync.dma_start(
                    out=a2a_in[cch * 128 : (cch + 1) * 128, :], in_=ctx_sb[:, csl])

        if DEBUG:
            nc.sync.dma_start(out=qTd[:, :], in_=qT)
            nc.sync.dma_start(out=kTd[:, :], in_=kT)
            nc.sync.dma_start(out=vvd[:, :], in_=vv.rearrange("p b c -> p (b c)"))
            nc.sync.dma_start(out=ctxd[:, :], in_=ctx_sb)

        # ---- A2A: head-sharded ctx -> token-sharded ctx ----
        nc.gpsimd.collective_compute(
            "AllToAll", mybir.AluOpType.bypass,
            replica_groups=[list(range(NC))],
            ins=[a2a_in.ap().opt()], outs=[a2a_out.ap().opt()])

        # ---- phase 3: out-projection on own token chunk ----
        with tc.tile_pool(name="ctxgp", bufs=1) as ctxgp, \
             tc.tile_pool(name="outp", bufs=3) as outp, \
             tc.tile_pool(name="ps6", bufs=2, space="PSUM") as ps6:
            ctxg = ctxgp.tile([128, 8, CHUNK], BF16, tag="ctxg")
            for tl in range(4):
                nc.scalar.dma_start(
                    out=ctxg[:, :, tl * 128 : (tl + 1) * 128],
                    in_=a2a_out.ap()[:, tl * 128 : (tl + 1) * 128]
                        .rearrange("(cb p) t -> p cb t", p=128))
            for tl in range(4):
                pso = {nh: ps6.tile([128, 512], F32, name=f"op{nh}", tag=f"op{nh}")
                       for nh in range(2)}
                for nh in range(2):
                    for cb in range(8):
                        nc.tensor.matmul(
                            pso[nh],
                            ctxg[:, cb, tl * 128 : (tl + 1) * 128],
                            wo_sb[:, cb, nh * 512 : (nh + 1) * 512],
                            start=(cb == 0), stop=(cb == 7))
                ost = outp.tile([128, 1024], F32, tag="ost")
                nc.scalar.copy(ost[:, 0:512], pso[0])
                nc.scalar.copy(ost[:, 512:1024], pso[1])
                nc.sync.dma_start(out=out_d[tl * 128 : (tl + 1) * 128, :], in_=ost)

    nc.compile()
    return nc


def _head_cols(h, deinterleave):
    base = h * DH
    if deinterleave:
        return np.concatenate([base + np.arange(0, DH, 2), base + np.arange(1, DH, 2)])
    return base + np.arange(DH)


def _make_tables():
    inv_freq = 1.0 / (THETA ** (np.arange(0, DH, 2) / DH))   # [32]
    ang = np.arange(S)[:, None] * inv_freq[None, :]          # [2048, 32]
    ch = np.cos(ang).T.astype(np.float32)                    # [32, 2048]
    sh = np.sin(ang).T.astype(np.float32)
    cosb = np.tile(np.concatenate([ch, ch, ch, ch], axis=0), (1, B))
    sinb = np.tile(np.concatenate([-sh, sh, -sh, sh], axis=0), (1, B))
    kk, qq = np.meshgrid(np.arange(128), np.arange(128), indexing="ij")
    dmask = (kk <= qq).astype(np.float32)
    bf = ml_dtypes.bfloat16
    return cosb.astype(bf), sinb.astype(bf), dmask.astype(bf)


def _in_maps(inputs):
    bf = ml_dtypes.bfloat16
    x = np.ascontiguousarray(inputs["x"], dtype=np.float32)
    norm_w = np.asarray(inputs["norm_w"], dtype=np.float32)
    wq = np.asarray(inputs["wq"], dtype=np.float32) * norm_w[:, None]
    wk = np.asarray(inputs["wk"], dtype=np.float32) * norm_w[:, None]
    wv = np.asarray(inputs["wv"], dtype=np.float32) * norm_w[:, None]
    wo = np.ascontiguousarray(inputs["wo"], dtype=np.float32).astype(bf)

    xT = np.ascontiguousarray(x.reshape(TOK, D).T.astype(bf))  # [1024, 4096]
    cosb, sinb, dmask = _make_tables()

    maps = []
    for c in range(NC):
        qcols = np.concatenate([_head_cols(2 * c, True), _head_cols(2 * c + 1, True)])
        vcols = np.concatenate([_head_cols(2 * c, False), _head_cols(2 * c + 1, False)])
        maps.append({
            "xc": np.ascontiguousarray(xT[:, c * CHUNK : (c + 1) * CHUNK]),
            "xtb": xT,
            "wqc": np.ascontiguousarray(wq[:, qcols].astype(bf)),
            "wkc": np.ascontiguousarray(wk[:, qcols].astype(bf)),
            "wvc": np.ascontiguousarray(wv[:, vcols].astype(bf)),
            "wo": wo,
            "cosb": cosb,
            "sinb": sinb,
            "dmask": dmask,
        })
    return maps


def _run(inputs, trace=False):
    if "nc" not in _CACHE:
        _CACHE["nc"] = _build()
    nc = _CACHE["nc"]
    res = run_bass_kernel_spmd(nc, _in_maps(inputs), core_ids=list(range(NC)),
                               trace=trace)
    chunks = [res.results[c]["out"] for c in range(NC)]
    out = np.concatenate(chunks, axis=0).reshape(B, S, D).astype(np.float32)
    return out, res


def kernel(**inputs) -> np.ndarray:
    out, _ = _run(inputs, trace=False)
    return out
